# revision 26
# baseline (speedup 1.0000x reference)
"""Transformer-XL relative attention kernel for 8 TRN2 NeuronCores.

Sharding: head-parallel. Core c owns heads (2c, 2c+1) for all 4 batches.
No collectives needed; host assembles/normalizes the outputs.

Per (batch, head) pair the device computes, in transposed layout:
  E2[i, c]   = exp(0.125 * (q+v)_i . rh_c)           (bf16, PE + ACT)
  pbuf       = E2 written to DRAM with row stride 1025, col 0 = 1.0 (pad)
  E2T[j, i]  = xbar-transpose read of pbuf flat[(i*1024 + 1024 + j)]
               == exp(0.125 * rel_shift(BD)[i, j]) transposed
  E1[j, i]   = exp(0.125 * K_j.(q+u)_i + maskbias_j)  (mask folded into ACT bias)
  num[j, i]  = E1 * E2T                               (softmax numerator^T)
  out_aug    = V_aug^T @ num  (V_aug has a ones column -> row 64 = denominator)
  out        = out_aug[0:64] * recip(denom)
Host: w[i,j] = num[j,i]*recip[i]; weight_mean = mean over 16 heads.
"""

import sys

sys.path.insert(0, "/opt/trn_rl_repo")

import numpy as np
import ml_dtypes


def _ensure_ntff_hook():
    """The agent image's antenv lacks axon_hooks; synthesize it and register
    the NTFF profile hook so run_bass_kernel_spmd(trace=True) can report
    exec_time_ns. Best-effort: failures only disable profiling."""
    try:
        import types

        try:
            import antenv.axon_hooks  # noqa: F401
            return
        except ImportError:
            pass
        import antenv

        mod = types.ModuleType("antenv.axon_hooks")
        _state = {"hook": None}
        mod.set_axon_ntff_profile_hook = lambda h: _state.__setitem__("hook", h)
        mod.get_axon_ntff_profile_hook = lambda: _state["hook"]
        sys.modules["antenv.axon_hooks"] = mod
        antenv.axon_hooks = mod
        from trn_agent_boot.trn_boot import _ntff_profile_via_ctypes

        hook = _ntff_profile_via_ctypes("/opt/axon/libaxon_pjrt.so")
        if hook is not None:
            mod.set_axon_ntff_profile_hook(hook)
    except Exception:
        pass


_ensure_ntff_hook()

import concourse.bass as bass
import concourse.mybir as mybir
import concourse.tile as tile
from concourse import bacc
from concourse.bass_utils import run_bass_kernel_spmd
from concourse.tile_rust import add_dep_helper

BF16 = mybir.dt.bfloat16
F32 = mybir.dt.float32
AF = mybir.ActivationFunctionType
ALU = mybir.AluOpType

S = 1024          # qlen == klen
B = 4             # batch
NH = 16           # heads
DH = 64           # head dim
NCORES = 8
HPC = NH // NCORES        # heads per core = 2
DSL = HPC * DH            # d_model slice per core = 128
MASK_BIAS = -60000.0      # exp(x + MASK_BIAS) == 0 in fp32 for any realistic x

_cached = {}


def _build():
    nc = bacc.Bacc("TRN2", target_bir_lowering=False, debug=False, num_devices=NCORES)

    qs = nc.declare_dram_parameter("qs", [S, B, DSL], BF16, isOutput=False)
    ks = nc.declare_dram_parameter("ks", [S, B, DSL], BF16, isOutput=False)
    vs = nc.declare_dram_parameter("vs", [S, B, DSL], BF16, isOutput=False)
    rr = nc.declare_dram_parameter("rr", [S, S], BF16, isOutput=False)
    wrs = nc.declare_dram_parameter("wrs", [DSL, S], BF16, isOutput=False)
    uvb = nc.declare_dram_parameter("uvb", [DSL, 3], F32, isOutput=False)
    mb = nc.declare_dram_parameter("mb", [S, B], F32, isOutput=False)

    out_o = nc.declare_dram_parameter("out_o", [B, HPC, 65, S], F32, isOutput=True)
    out_w = nc.declare_dram_parameter("out_w", [B, HPC, S, S], BF16, isOutput=True)

    # per-pair padded bounce buffers for the rel-shift (bf16, row stride S+1)
    pbuf = nc.dram_tensor("pbuf", [B * HPC, S, S + 1], BF16)
    pb = pbuf.ap()
    PAIR_ELEMS = S * (S + 1)

    with tile.TileContext(nc) as tc:
        with (
            tc.tile_pool(name="singles", bufs=1) as singles,
            tc.tile_pool(name="bpool", bufs=2) as bpool,
            tc.tile_pool(name="vpool", bufs=2) as vpool,
            tc.tile_pool(name="e2pool", bufs=3) as e2pool,
            tc.tile_pool(name="e2tpool", bufs=3) as e2tpool,
            tc.tile_pool(name="e1pool", bufs=3) as e1pool,
            tc.tile_pool(name="numpool", bufs=3) as numpool,
            tc.tile_pool(name="opool", bufs=2) as opool,
            tc.tile_pool(name="psbd", bufs=2, space="PSUM") as psbd,
            tc.tile_pool(name="pssc", bufs=2, space="PSUM") as pssc,
            tc.tile_pool(name="psav", bufs=2, space="PSUM") as psav,
        ):
            # ---- one-time setup ----
            # u/v/br columns (host passes [DSL, 3])
            uvbT = singles.tile([128, 3], F32)
            nc.sync.dma_start(out=uvbT, in_=uvb.ap())
            u_col = uvbT[:, 0:1]
            v_col = uvbT[:, 1:2]
            br_col = uvbT[:, 2:3]

            # mask bias: mbT[j, jt, b] = mb[jt*128+j, b] (host passes [S, B])
            mbT = singles.tile([128, 8, B], F32)
            nc.sync.dma_start(
                out=mbT, in_=mb.ap().rearrange("(jt j) b -> j jt b", j=128)
            )

            # Wr^T and r^T via xbar transpose loads (bf16)
            wrT = singles.tile([128, 8, DSL], BF16)
            for ct in range(8):
                nc.sync.dma_start(
                    out=wrT[:, ct, :],
                    in_=wrs.ap()[:, ct * 128:(ct + 1) * 128],
                    transpose=True,
                )
            rT = singles.tile([128, 8, S], BF16)
            for ct in range(8):
                nc.sync.dma_start(
                    out=rT[:, ct, :],
                    in_=rr.ap()[:, ct * 128:(ct + 1) * 128],
                    transpose=True,
                )

            # rh^T = Wr_slice @ r^T + br  -> [128 (2 heads x 64), 1024] bf16
            rhT = singles.tile([128, S], BF16)
            for ch in range(2):
                rh_ps = pssc.tile([128, 512], F32, tag="sc")
                for ct in range(8):
                    nc.tensor.matmul(
                        rh_ps,
                        lhsT=wrT[:, ct, :],
                        rhs=rT[:, ct, ch * 512:(ch + 1) * 512],
                        start=(ct == 0),
                        stop=(ct == 7),
                    )
                nc.vector.tensor_scalar_add(
                    rhT[:, ch * 512:(ch + 1) * 512], rh_ps, br_col
                )

            # ---- main loops (software-pipelined: phase1(p+1) before phase2(p)) ----
            bctx = {}

            def prep_b(b):
                qT = bpool.tile([128, S], BF16, tag="qT")
                nc.sync.dma_start(out=qT, in_=qs.ap()[:, b, :], transpose=True)
                kT = bpool.tile([128, S], BF16, tag="kT")
                nc.sync.dma_start(out=kT, in_=ks.ap()[:, b, :], transpose=True)
                quT = bpool.tile([128, S], BF16, tag="quT")
                nc.vector.tensor_scalar_add(quT, qT, u_col)
                qvT = bpool.tile([128, S], BF16, tag="qvT")
                nc.vector.tensor_scalar_add(qvT, qT, v_col)
                vaug = vpool.tile([128, HPC * 8, 65], BF16, tag="vaug")
                nc.vector.memset(vaug[:, :, 64:65], 1.0)
                for jt in range(8):
                    nc.gpsimd.dma_start(
                        out=vaug[:, jt::8, 0:64],
                        in_=vs.ap()[jt * 128:(jt + 1) * 128, b, :].rearrange(
                            "p (h d) -> p h d", h=HPC
                        ),
                    )
                bctx[b] = (kT, quT, qvT, vaug)

            def phase1(pair):
                b, h = divmod(pair, HPC)
                hs = slice(h * DH, (h + 1) * DH)
                qvT = bctx[b][2]
                pbuf_writes = []
                for it in range(8):
                    e2 = e2pool.tile([128, S + 1], BF16, tag="e2")
                    nc.vector.memset(e2[:, 0:1], 1.0)
                    for ch in range(2):
                        bd_ps = psbd.tile([128, 512], F32, tag="bd")
                        nc.tensor.matmul(
                            bd_ps,
                            lhsT=qvT[hs, it * 128:(it + 1) * 128],
                            rhs=rhT[hs, ch * 512:(ch + 1) * 512],
                            start=True,
                            stop=True,
                        )
                        nc.scalar.activation(
                            out=e2[:, 1 + ch * 512:1 + (ch + 1) * 512],
                            in_=bd_ps,
                            func=AF.Exp,
                            scale=0.125,
                        )
                    winst = nc.sync.dma_start(
                        out=pb[pair, it * 128:(it + 1) * 128, :], in_=e2
                    )
                    pbuf_writes.append(winst.ins)
                return pbuf_writes

            def phase2(pair, pbuf_writes):
                b, h = divmod(pair, HPC)
                hs = slice(h * DH, (h + 1) * DH)
                kT, quT, _, vaug = bctx[b]
                av_ps = psav.tile([65, S], F32, tag="av")
                for jt in range(8):
                    e2t = e2tpool.tile([128, S], BF16, tag="e2t")
                    src = bass.AP(
                        tensor=pb.tensor,
                        offset=pair * PAIR_ELEMS + S + jt * 128,
                        ap=[[S, S], [1, 128]],
                    )
                    rinst = nc.sync.dma_start(out=e2t, in_=src, transpose=True)
                    for w in pbuf_writes:
                        add_dep_helper(rinst.ins, w, reason="pbuf write->tread")

                    e1 = e1pool.tile([128, S], BF16, tag="e1")
                    for ch in range(2):
                        sc_ps = pssc.tile([128, 512], F32, tag="sc")
                        nc.tensor.matmul(
                            sc_ps,
                            lhsT=kT[hs, jt * 128:(jt + 1) * 128],
                            rhs=quT[hs, ch * 512:(ch + 1) * 512],
                            start=True,
                            stop=True,
                        )
                        nc.scalar.activation(
                            out=e1[:, ch * 512:(ch + 1) * 512],
                            in_=sc_ps,
                            func=AF.Exp,
                            bias=mbT[:, jt, b:b + 1],
                            scale=0.125,
                        )
                    num = numpool.tile([128, S], BF16, tag="num")
                    nc.vector.tensor_tensor(num, e1, e2t, op=ALU.mult)
                    nc.gpsimd.dma_start(
                        out=out_w.ap()[b, h, jt * 128:(jt + 1) * 128, :], in_=num
                    )
                    for ch in range(2):
                        nc.tensor.matmul(
                            av_ps[:, ch * 512:(ch + 1) * 512],
                            lhsT=vaug[:, h * 8 + jt, :],
                            rhs=num[:, ch * 512:(ch + 1) * 512],
                            start=(jt == 0),
                            stop=(jt == 7),
                        )
                oo = opool.tile([65, S], F32, tag="oo")
                nc.vector.tensor_copy(oo, av_ps)
                nc.gpsimd.dma_start(out=out_o.ap()[b, h, :, :], in_=oo)

            NP = B * HPC
            prep_b(0)
            pw = {0: phase1(0)}
            for p in range(NP):
                nxt = p + 1
                if nxt < NP:
                    if nxt % HPC == 0:
                        prep_b(nxt // HPC)
                    pw[nxt] = phase1(nxt)
                phase2(p, pw.pop(p))

    nc.compile()
    return nc


def _get_nc():
    if "nc" not in _cached:
        _cached["nc"] = _build()
    return _cached["nc"]


def kernel(q, K, V, mask, r, u, v, Wr, br):
    nc = _get_nc()
    bf = ml_dtypes.bfloat16

    q = np.asarray(q, np.float32)
    K = np.asarray(K, np.float32)
    V = np.asarray(V, np.float32)
    r = np.asarray(r, np.float32)
    u = np.asarray(u, np.float32)
    v = np.asarray(v, np.float32)
    Wr = np.asarray(Wr, np.float32)
    br = np.asarray(br, np.float32)
    mb_host = np.ascontiguousarray(
        np.where(np.asarray(mask, bool)[0, :, :, 0], MASK_BIAS, 0.0).astype(np.float32)
    )  # (S, B)
    rr_host = r.astype(bf)

    in_maps = []
    for c in range(NCORES):
        dsl = slice(c * DSL, (c + 1) * DSL)
        uvb_host = np.ascontiguousarray(
            np.stack(
                [
                    u[c * HPC:(c + 1) * HPC].reshape(-1),
                    v[c * HPC:(c + 1) * HPC].reshape(-1),
                    br[dsl],
                ]
            ).T
        ).astype(np.float32)  # (DSL, 3)
        in_maps.append(
            {
                "qs": np.ascontiguousarray(q[:, :, dsl]).astype(bf),
                "ks": np.ascontiguousarray(K[:, :, dsl]).astype(bf),
                "vs": np.ascontiguousarray(V[:, :, dsl]).astype(bf),
                "rr": rr_host,
                "wrs": np.ascontiguousarray(Wr[dsl, :]).astype(bf),
                "uvb": uvb_host,
                "mb": mb_host,
            }
        )

    res = run_bass_kernel_spmd(nc, in_maps, core_ids=list(range(NCORES)))
    kernel._last_exec_ns = res.exec_time_ns
    results = res.results

    out = np.empty((S, B, NH * DH), np.float32)
    wmean = np.zeros((S, S, B), np.float32)
    for c in range(NCORES):
        ro = np.asarray(results[c]["out_o"], np.float32)  # (B, HPC, 65, S)
        rw = np.asarray(results[c]["out_w"])  # (B, HPC, S, S) bf16 [j, i]
        recips = 1.0 / ro[:, :, 64, :]  # (B, HPC, S)
        for b in range(B):
            for h in range(HPC):
                dsl2 = slice(c * DSL + h * DH, c * DSL + (h + 1) * DH)
                out[:, b, dsl2] = (ro[b, h, 0:64, :] * recips[b, h][None, :]).T
                wmean[:, :, b] += (
                    rw[b, h].astype(np.float32) * recips[b, h][None, :]
                ).T
    wmean *= 1.0 / NH
    return out, wmean


# revision 27
# speedup vs baseline: 1.1334x; 1.1334x over previous
"""Transformer-XL relative attention kernel for 8 TRN2 NeuronCores.

Sharding: head-parallel. Core c owns heads (2c, 2c+1) for all 4 batches.
No collectives needed; host assembles/normalizes the outputs.

Per (batch, head) pair the device computes, in transposed layout:
  E2[i, c]   = exp(0.125 * (q+v)_i . rh_c)           (bf16, PE + ACT)
  pbuf       = E2 written to DRAM with row stride 1025, col 0 = 1.0 (pad)
  E2T[j, i]  = xbar-transpose read of pbuf flat[(i*1024 + 1024 + j)]
               == exp(0.125 * rel_shift(BD)[i, j]) transposed
  E1[j, i]   = exp(0.125 * K_j.(q+u)_i + maskbias_j)  (mask folded into ACT bias)
  num[j, i]  = E1 * E2T                               (softmax numerator^T)
  out_aug    = V_aug^T @ num  (V_aug has a ones column -> row 64 = denominator)
  out        = out_aug[0:64] * recip(denom)
Host: w[i,j] = num[j,i]*recip[i]; weight_mean = mean over 16 heads.
"""

import sys

sys.path.insert(0, "/opt/trn_rl_repo")

import numpy as np
import ml_dtypes


def _ensure_ntff_hook():
    """The agent image's antenv lacks axon_hooks; synthesize it and register
    the NTFF profile hook so run_bass_kernel_spmd(trace=True) can report
    exec_time_ns. Best-effort: failures only disable profiling."""
    try:
        import types

        try:
            import antenv.axon_hooks  # noqa: F401
            return
        except ImportError:
            pass
        import antenv

        mod = types.ModuleType("antenv.axon_hooks")
        _state = {"hook": None}
        mod.set_axon_ntff_profile_hook = lambda h: _state.__setitem__("hook", h)
        mod.get_axon_ntff_profile_hook = lambda: _state["hook"]
        sys.modules["antenv.axon_hooks"] = mod
        antenv.axon_hooks = mod
        from trn_agent_boot.trn_boot import _ntff_profile_via_ctypes

        hook = _ntff_profile_via_ctypes("/opt/axon/libaxon_pjrt.so")
        if hook is not None:
            mod.set_axon_ntff_profile_hook(hook)
    except Exception:
        pass


_ensure_ntff_hook()

import concourse.bass as bass
import concourse.mybir as mybir
import concourse.tile as tile
from concourse import bacc
from concourse.bass_utils import run_bass_kernel_spmd
from concourse.tile_rust import add_dep_helper

BF16 = mybir.dt.bfloat16
F32 = mybir.dt.float32
AF = mybir.ActivationFunctionType
ALU = mybir.AluOpType

S = 1024          # qlen == klen
B = 4             # batch
NH = 16           # heads
DH = 64           # head dim
NCORES = 8
HPC = NH // NCORES        # heads per core = 2
DSL = HPC * DH            # d_model slice per core = 128
MASK_BIAS = -60000.0      # exp(x + MASK_BIAS) == 0 in fp32 for any realistic x

_cached = {}


def _build():
    nc = bacc.Bacc("TRN2", target_bir_lowering=False, debug=False, num_devices=NCORES)

    qs = nc.declare_dram_parameter("qs", [S, B, DSL], BF16, isOutput=False)
    ks = nc.declare_dram_parameter("ks", [S, B, DSL], BF16, isOutput=False)
    vs = nc.declare_dram_parameter("vs", [S, B, DSL], BF16, isOutput=False)
    rr = nc.declare_dram_parameter("rr", [S, S], BF16, isOutput=False)
    wrs = nc.declare_dram_parameter("wrs", [DSL, S], BF16, isOutput=False)
    uvb = nc.declare_dram_parameter("uvb", [DSL, 3], F32, isOutput=False)
    mb = nc.declare_dram_parameter("mb", [S, B], F32, isOutput=False)

    out_o = nc.declare_dram_parameter("out_o", [B, HPC, 65, S], F32, isOutput=True)
    out_w = nc.declare_dram_parameter("out_w", [B, HPC, S, S], BF16, isOutput=True)

    # per-pair padded bounce buffers for the rel-shift (bf16, row stride S+1)
    pbuf = nc.dram_tensor("pbuf", [B * HPC, S, S + 1], BF16)
    pb = pbuf.ap()
    PAIR_ELEMS = S * (S + 1)

    with tile.TileContext(nc) as tc:
        with (
            tc.tile_pool(name="singles", bufs=1) as singles,
            tc.tile_pool(name="bpool", bufs=2) as bpool,
            tc.tile_pool(name="vpool", bufs=2) as vpool,
            tc.tile_pool(name="e2pool", bufs=3) as e2pool,
            tc.tile_pool(name="e2tpool", bufs=2) as e2tpool,
            tc.tile_pool(name="e1pool", bufs=2) as e1pool,
            tc.tile_pool(name="numpool", bufs=3) as numpool,
            tc.tile_pool(name="opool", bufs=2) as opool,
            tc.tile_pool(name="psbig", bufs=2, space="PSUM") as psbig,
            tc.tile_pool(name="psav", bufs=2, space="PSUM") as psav,
        ):
            # ---- one-time setup ----
            # u/v/br columns (host passes [DSL, 3])
            uvbT = singles.tile([128, 3], F32)
            nc.sync.dma_start(out=uvbT, in_=uvb.ap())
            u_col = uvbT[:, 0:1]
            v_col = uvbT[:, 1:2]
            br_col = uvbT[:, 2:3]

            # mask bias: mbT[j, jt, b] = mb[jt*128+j, b] (host passes [S, B])
            mbT = singles.tile([128, 8, B], F32)
            nc.sync.dma_start(
                out=mbT, in_=mb.ap().rearrange("(jt j) b -> j jt b", j=128)
            )

            # Wr^T and r^T via xbar transpose loads (bf16)
            wrT = singles.tile([128, 8, DSL], BF16)
            for ct in range(8):
                nc.sync.dma_start(
                    out=wrT[:, ct, :],
                    in_=wrs.ap()[:, ct * 128:(ct + 1) * 128],
                    transpose=True,
                )
            rT = singles.tile([128, 8, S], BF16)
            for ct in range(8):
                nc.sync.dma_start(
                    out=rT[:, ct, :],
                    in_=rr.ap()[:, ct * 128:(ct + 1) * 128],
                    transpose=True,
                )

            # rh^T = Wr_slice @ r^T + br  -> [128 (2 heads x 64), 1024] bf16
            rhT = singles.tile([128, S], BF16)
            for ch in range(2):
                rh_ps = psbig.tile([128, 512], F32, tag="score")
                for ct in range(8):
                    nc.tensor.matmul(
                        rh_ps,
                        lhsT=wrT[:, ct, :],
                        rhs=rT[:, ct, ch * 512:(ch + 1) * 512],
                        start=(ct == 0),
                        stop=(ct == 7),
                    )
                nc.vector.tensor_scalar_add(
                    rhT[:, ch * 512:(ch + 1) * 512], rh_ps, br_col
                )

            # ---- main loops ----
            for b in range(B):
                qT = bpool.tile([128, S], BF16, tag="qT")
                nc.sync.dma_start(out=qT, in_=qs.ap()[:, b, :], transpose=True)
                kT = bpool.tile([128, S], BF16, tag="kT")
                nc.sync.dma_start(out=kT, in_=ks.ap()[:, b, :], transpose=True)

                quT = bpool.tile([128, S], BF16, tag="quT")
                nc.vector.tensor_scalar_add(quT, qT, u_col)
                qvT = bpool.tile([128, S], BF16, tag="qvT")
                nc.vector.tensor_scalar_add(qvT, qT, v_col)

                # V with ones column: [128 j, (h, jt), 65]
                vaug = vpool.tile([128, HPC * 8, 65], BF16, tag="vaug")
                nc.vector.memset(vaug[:, :, 64:65], 1.0)
                for jt in range(8):
                    nc.sync.dma_start(
                        out=vaug[:, jt::8, 0:64],
                        in_=vs.ap()[jt * 128:(jt + 1) * 128, b, :].rearrange(
                            "p (h d) -> p h d", h=HPC
                        ),
                    )

                for h in range(HPC):
                    pair = b * HPC + h
                    hs = slice(h * DH, (h + 1) * DH)

                    # ---- phase 1: E2 = exp(BD_raw/8) -> padded DRAM ----
                    pbuf_writes = []
                    for it in range(8):
                        bd_ps = psbig.tile([128, S], F32, tag="score")
                        for ch in range(2):
                            nc.tensor.matmul(
                                bd_ps[:, ch * 512:(ch + 1) * 512],
                                lhsT=qvT[hs, it * 128:(it + 1) * 128],
                                rhs=rhT[hs, ch * 512:(ch + 1) * 512],
                                start=True,
                                stop=True,
                            )
                        e2 = e2pool.tile([128, S + 1], BF16, tag="e2")
                        nc.vector.memset(e2[:, 0:1], 1.0)
                        nc.scalar.activation(
                            out=e2[:, 1:S + 1], in_=bd_ps, func=AF.Exp, scale=0.125
                        )
                        winst = nc.sync.dma_start(
                            out=pb[pair, it * 128:(it + 1) * 128, :], in_=e2
                        )
                        pbuf_writes.append(winst.ins)

                    # ---- phase 2: per key tile ----
                    av_ps = psav.tile([65, S], F32, tag="av")
                    for jt in range(8):
                        e2t = e2tpool.tile([128, S], BF16, tag="e2t")
                        src = bass.AP(
                            tensor=pb.tensor,
                            offset=pair * PAIR_ELEMS + S + jt * 128,
                            ap=[[S, S], [1, 128]],
                        )
                        rinst = nc.sync.dma_start(out=e2t, in_=src, transpose=True)
                        for w in pbuf_writes:
                            add_dep_helper(rinst.ins, w, reason="pbuf write->tread")

                        sc_ps = psbig.tile([128, S], F32, tag="score")
                        for ch in range(2):
                            nc.tensor.matmul(
                                sc_ps[:, ch * 512:(ch + 1) * 512],
                                lhsT=kT[hs, jt * 128:(jt + 1) * 128],
                                rhs=quT[hs, ch * 512:(ch + 1) * 512],
                                start=True,
                                stop=True,
                            )
                        e1 = e1pool.tile([128, S], BF16, tag="e1")
                        nc.scalar.activation(
                            out=e1,
                            in_=sc_ps,
                            func=AF.Exp,
                            bias=mbT[:, jt, b:b + 1],
                            scale=0.125,
                        )
                        num = numpool.tile([128, S], BF16, tag="num")
                        nc.vector.tensor_tensor(num, e1, e2t, op=ALU.mult)
                        nc.sync.dma_start(
                            out=out_w.ap()[b, h, jt * 128:(jt + 1) * 128, :], in_=num
                        )
                        for ch in range(2):
                            nc.tensor.matmul(
                                av_ps[:, ch * 512:(ch + 1) * 512],
                                lhsT=vaug[:, h * 8 + jt, :],
                                rhs=num[:, ch * 512:(ch + 1) * 512],
                                start=(jt == 0),
                                stop=(jt == 7),
                            )

                    # ---- epilogue: raw (unnormalized) output + denominator row ----
                    oo = opool.tile([65, S], F32, tag="oo")
                    nc.vector.tensor_copy(oo, av_ps)
                    nc.sync.dma_start(out=out_o.ap()[b, h, :, :], in_=oo)

    nc.compile()
    return nc


def _get_nc():
    if "nc" not in _cached:
        _cached["nc"] = _build()
    return _cached["nc"]


def kernel(q, K, V, mask, r, u, v, Wr, br):
    nc = _get_nc()
    bf = ml_dtypes.bfloat16

    q = np.asarray(q, np.float32)
    K = np.asarray(K, np.float32)
    V = np.asarray(V, np.float32)
    r = np.asarray(r, np.float32)
    u = np.asarray(u, np.float32)
    v = np.asarray(v, np.float32)
    Wr = np.asarray(Wr, np.float32)
    br = np.asarray(br, np.float32)
    mb_host = np.ascontiguousarray(
        np.where(np.asarray(mask, bool)[0, :, :, 0], MASK_BIAS, 0.0).astype(np.float32)
    )  # (S, B)
    rr_host = r.astype(bf)

    in_maps = []
    for c in range(NCORES):
        dsl = slice(c * DSL, (c + 1) * DSL)
        uvb_host = np.ascontiguousarray(
            np.stack(
                [
                    u[c * HPC:(c + 1) * HPC].reshape(-1),
                    v[c * HPC:(c + 1) * HPC].reshape(-1),
                    br[dsl],
                ]
            ).T
        ).astype(np.float32)  # (DSL, 3)
        in_maps.append(
            {
                "qs": np.ascontiguousarray(q[:, :, dsl]).astype(bf),
                "ks": np.ascontiguousarray(K[:, :, dsl]).astype(bf),
                "vs": np.ascontiguousarray(V[:, :, dsl]).astype(bf),
                "rr": rr_host,
                "wrs": np.ascontiguousarray(Wr[dsl, :]).astype(bf),
                "uvb": uvb_host,
                "mb": mb_host,
            }
        )

    res = run_bass_kernel_spmd(nc, in_maps, core_ids=list(range(NCORES)))
    kernel._last_exec_ns = res.exec_time_ns
    results = res.results

    out = np.empty((S, B, NH * DH), np.float32)
    wmean = np.zeros((S, S, B), np.float32)
    for c in range(NCORES):
        ro = np.asarray(results[c]["out_o"], np.float32)  # (B, HPC, 65, S)
        rw = np.asarray(results[c]["out_w"])  # (B, HPC, S, S) bf16 [j, i]
        recips = 1.0 / ro[:, :, 64, :]  # (B, HPC, S)
        for b in range(B):
            for h in range(HPC):
                dsl2 = slice(c * DSL + h * DH, c * DSL + (h + 1) * DH)
                out[:, b, dsl2] = (ro[b, h, 0:64, :] * recips[b, h][None, :]).T
                wmean[:, :, b] += (
                    rw[b, h].astype(np.float32) * recips[b, h][None, :]
                ).T
    wmean *= 1.0 / NH
    return out, wmean
